# revision 7
# baseline (speedup 1.0000x reference)
"""MicroHeadAttention Trainium2 kernel (8-core SPMD, data-parallel over
(batch, row-chunk) pairs).

Shapes (hardcoded): x (2, 2048, 1024), weights (1024, 1024), biases (1024,).
EMBED=1024, 16 heads in 2 blocks (g) of 8 micro-heads, head_dim 64.

Decomposition: the reference's "scramble" is a raw row-major reshape, so the
attention head (b, g, m') consumes exactly rows x[b, 256m':256(m'+1)] and
weight columns [512g:512(g+1)], reshaped (256, 512) -> (2048, 64) with
scrambled position n' = 8*row + m (m = 64-channel sub-block).  16 (b, m')
row-chunks across 8 cores = 2 per core; each chunk has g=0,1 -> 4 heads/core.

Per-core dataflow:
  phase 1: V = x@Wv.T+bv (natural row-major), scrambled to (n', d) layout via
           a DRAM round-trip DMA (with a ones-column appended for the softmax
           denominator).  Q^T/K^T computed channels-on-partitions; the
           mandatory PSUM->SBUF bias copies write strided so qsc/ksc come out
           directly in scrambled (d, n') layout, g on partition halves.
  phase 2: per head, per 512-wide q block: S^T = k^T.T @ q^T (K=64 matmuls,
           g=0/g=1 at partition bases 0/64 -> concurrent PE row-groups),
           causal masks added only on the 4 diagonal blocks, exp on ACT
           (no max subtraction; |S| < ~3), ctx^T accumulated as
           [v | ones].T @ P^T giving ctx rows 0..64 and denominator row 64.
  phase 3: out = ctx^T.T @ Wo^T + bo in natural row layout, ctx^T consumed
           directly with stride-8 lhsT access patterns; division by the
           denominator folded into the ctx^T copies.
"""

import numpy as np

import concourse.bass as bass
import concourse.mybir as mybir
from concourse import bacc
from concourse.tile import TileContext
from concourse.bass_utils import run_bass_kernel_spmd

F32 = mybir.dt.float32
FR = mybir.dt.float32r  # full-rate fp32 matmul dtype (TF32-like rounding)
DT_MM = FR
NEG = -1e30
E = 1024
R = 512       # rows per core
RP = 256      # rows per pair
ALU = mybir.AluOpType
ACTF = mybir.ActivationFunctionType

_cache = {}


def _build():
    nc = bacc.Bacc()
    xT_d = nc.dram_tensor("xT", (E, R), F32, kind="ExternalInput")
    wq_d = nc.dram_tensor("wqT", (E, E), F32, kind="ExternalInput")
    wk_d = nc.dram_tensor("wkT", (E, E), F32, kind="ExternalInput")
    wv_d = nc.dram_tensor("wvT", (E, E), F32, kind="ExternalInput")
    wo_d = nc.dram_tensor("woTre", (128, 8, E), F32, kind="ExternalInput")
    bq_d = nc.dram_tensor("bqT", (128, 8), F32, kind="ExternalInput")
    bk_d = nc.dram_tensor("bkT8", (128, 8), F32, kind="ExternalInput")
    bv_d = nc.dram_tensor("bvrow", (1, E), F32, kind="ExternalInput")
    bo_d = nc.dram_tensor("borow", (1, E), F32, kind="ExternalInput")
    out_d = nc.dram_tensor("out", (R, E), F32, kind="ExternalOutput")

    with TileContext(nc) as tc:
        with (
            tc.tile_pool(name="persist", bufs=1) as pp,
            tc.tile_pool(name="pt", bufs=4) as ptp,
            tc.tile_pool(name="misc", bufs=2) as mp,
            tc.tile_pool(name="ps", bufs=5, space="PSUM") as psp,
            tc.tile_pool(name="psctx", bufs=3, space="PSUM") as pcp,
            tc.tile_pool(name="dram", bufs=1, space="DRAM") as dp,
        ):
            # ---- persistent tiles ----
            bqT = pp.tile([128, 8], F32, tag="bqT", name="bqT")
            bkT8 = pp.tile([128, 8], F32, tag="bkT8", name="bkT8")
            bvr = pp.tile([1, E], F32, tag="bvr", name="bvr")
            bor = pp.tile([1, E], F32, tag="bor", name="bor")
            bv_bc = pp.tile([128, E], F32, tag="bvbc", name="bvbc")
            bo_bc = pp.tile([128, E], F32, tag="bobc", name="bobc")
            masks = pp.tile([128, 4, 512], F32, tag="masks", name="masks")
            qsc = [pp.tile([128, 2048], DT_MM, tag=f"qsc{p}", name=f"qsc{p}") for p in range(2)]
            ksc = [pp.tile([128, 2048], DT_MM, tag=f"ksc{p}", name=f"ksc{p}") for p in range(2)]
            vsc = [[pp.tile([128, 16, 65], DT_MM, tag=f"vsc{p}{g}", name=f"vsc{p}{g}") for g in range(2)]
                   for p in range(2)]
            # ctxP[c=64g+d, rc, m, r]: contiguous lhsT slices for the out-proj
            ctxP = [pp.tile([128, 2, 8, 128], DT_MM, tag=f"ctxP{p}", name=f"ctxP{p}") for p in range(2)]
            vtmp = dp.tile([2, 2, 2048, 64], DT_MM, tag="vtmp", name="vtmp")

            nc.sync.dma_start(bqT[:], bq_d[:])
            nc.sync.dma_start(bkT8[:], bk_d[:])
            nc.sync.dma_start(bvr[:], bv_d[:])
            nc.sync.dma_start(bor[:], bo_d[:])
            nc.gpsimd.partition_broadcast(bv_bc[:], bvr[:])
            nc.gpsimd.partition_broadcast(bo_bc[:], bor[:])
            for o in range(4):
                m = masks[:, o, :]
                nc.gpsimd.memset(m, 0.0)
                nc.gpsimd.affine_select(
                    out=m, in_=m, compare_op=ALU.is_ge, fill=NEG,
                    base=-(128 * o), pattern=[[1, 512]], channel_multiplier=-1)
            # ones for the appended denominator column of each vsc tile
            # (memset cannot write fp32r; go through an f32 staging tile)
            ones16 = pp.tile([128, 16], F32, tag="ones16", name="ones16")
            nc.gpsimd.memset(ones16[:], 1.0)
            for p in range(2):
                for g in range(2):
                    nc.vector.tensor_copy(vsc[p][g][:, :, 64], ones16[:])

            with tc.tile_pool(name="stage1", bufs=1) as s1p:
                xt = s1p.tile([128, 8, R], DT_MM, tag="xt", name="xt")
                wq = s1p.tile([128, 8, E], DT_MM, tag="wq", name="wq")
                nc.sync.dma_start(xt[:], xT_d.rearrange("(ko ki) r -> ki ko r", ki=128).bitcast(DT_MM))
                nc.sync.dma_start(wq[:], wq_d.rearrange("(ko ki) o -> ki ko o", ki=128).bitcast(DT_MM))

                def qk_proj(w_tile, bias_tile, scale, dst):
                    for t in range(8):
                        ps = psp.tile([128, 512], F32, tag="psA", name="psA")
                        for ki in range(8):
                            nc.tensor.matmul(
                                ps[:], w_tile[:, ki, 128 * t:128 * (t + 1)],
                                xt[:, ki, :], start=(ki == 0), stop=(ki == 7))
                        g, u = t // 4, t % 4
                        for p in range(2):
                            for mh in range(2):
                                mm = 2 * u + mh
                                dest = dst[p].rearrange("c (r m) -> c r m", m=8)[
                                    64 * g:64 * (g + 1), :, mm]
                                nc.scalar.activation(
                                    dest, ps[64 * mh:64 * (mh + 1), RP * p:RP * (p + 1)],
                                    ACTF.Identity,
                                    bias=bias_tile[64 * mh:64 * (mh + 1), t:t + 1],
                                    scale=scale)

                with tc.tile_pool(name="stagev", bufs=1) as svp:
                    wv = svp.tile([128, 8, E], DT_MM, tag="wv", name="wv")
                    vnat = [svp.tile([128, 2, E], DT_MM, tag=f"vnat{p}", name=f"vnat{p}") for p in range(2)]
                    nc.sync.dma_start(wv[:], wv_d.rearrange("(ko ki) o -> ki ko o", ki=128).bitcast(DT_MM))

                    for rc in range(4):
                        p, half = rc // 2, rc % 2
                        for oc in range(2):
                            ps = psp.tile([128, 512], F32, tag="psA", name="psA")
                            for ki in range(8):
                                nc.tensor.matmul(
                                    ps[:], xt[:, ki, 128 * rc:128 * (rc + 1)],
                                    wv[:, ki, 512 * oc:512 * (oc + 1)],
                                    start=(ki == 0), stop=(ki == 7))
                            nc.vector.tensor_tensor(
                                vnat[p][:, half, 512 * oc:512 * (oc + 1)],
                                ps[:], bv_bc[:, 512 * oc:512 * (oc + 1)], ALU.add)
                    for p in range(2):
                        for g in range(2):
                            src = vnat[p][:, :, 512 * g:512 * (g + 1)].rearrange(
                                "r h (m d) -> r h m d", m=8)
                            dst = vtmp[p, g].rearrange(
                                "(h r m) d -> r h m d", h=2, r=128, m=8)
                            nc.sync.dma_start(dst, src)

                    # Q projection overlaps with V scramble DMAs
                    qk_proj(wq, bqT, 1.0, qsc)

                # wv/vnat freed; wk reuses that space
                with tc.tile_pool(name="stagek", bufs=1) as skp:
                    wk = skp.tile([128, 8, E], DT_MM, tag="wk", name="wk")
                    nc.sync.dma_start(wk[:], wk_d.rearrange("(ko ki) o -> ki ko o", ki=128).bitcast(DT_MM))
                    # V scrambled read-back (vtmp -> vsc)
                    for p in range(2):
                        for g in range(2):
                            nc.sync.dma_start(
                                vsc[p][g][:, :, 0:64],
                                vtmp[p, g].rearrange("(kb pin) d -> pin kb d", pin=128))
                    qk_proj(wk, bkT8, 0.125, ksc)

            # stage1 (xt, wq) freed; wo loads into that space
            with tc.tile_pool(name="stageo", bufs=1) as sop:
                wo = sop.tile([128, 8, E], DT_MM, tag="wo", name="wo")
                nc.sync.dma_start(wo[:], wo_d[:].bitcast(DT_MM))

                # ---- attention ----
                for p in range(2):
                    for j5 in range(4):
                        nkb = 4 * (j5 + 1)
                        ctx_ps = [pcp.tile([65, 512], F32, tag="ctxps", name="ctxps") for _ in range(2)]
                        for kb in range(nkb):
                            pts = []
                            for g in range(2):
                                st = psp.tile([128, 512], F32, tag="psA", name="psA")
                                nc.tensor.matmul(
                                    st[:],
                                    ksc[p][64 * g:64 * (g + 1), 128 * kb:128 * (kb + 1)],
                                    qsc[p][64 * g:64 * (g + 1), 512 * j5:512 * (j5 + 1)],
                                    start=True, stop=True)
                                if kb >= 4 * j5:
                                    nc.vector.tensor_tensor(
                                        st[:], st[:], masks[:, kb - 4 * j5, :], ALU.add)
                                pt = ptp.tile([128, 512], DT_MM, tag="pt", name="pt")
                                nc.scalar.activation(pt[:], st[:], ACTF.Exp)
                                pts.append(pt)
                            for g in range(2):
                                nc.tensor.matmul(
                                    ctx_ps[g][:], vsc[p][g][:, kb, :], pts[g][:],
                                    start=(kb == 0), stop=(kb == nkb - 1))
                        for g in range(2):
                            rec = mp.tile([1, 512], F32, tag="rec", name="rec")
                            nc.vector.reciprocal(rec[:], ctx_ps[g][64:65, :])
                            rbc = mp.tile([64, 512], F32, tag="rbc", name="rbc")
                            nc.gpsimd.partition_broadcast(rbc[:], rec[:])
                            # scatter (q = 8r + m) into (m, r) proj layout
                            dest = ctxP[p][64 * g:64 * (g + 1), j5 // 2, :,
                                           64 * (j5 % 2):64 * (j5 % 2) + 64]
                            nc.vector.tensor_tensor(
                                dest,
                                ctx_ps[g][0:64, :].rearrange("c (r m) -> c m r", m=8),
                                rbc[:].rearrange("c (r m) -> c m r", m=8),
                                ALU.mult)

                # ---- output projection ----
                for p in range(2):
                    ctx_v = ctxP[p]
                    for rc in range(2):
                        for oc in range(2):
                            ps = psp.tile([128, 512], F32, tag="psA", name="psA")
                            for mm in range(8):
                                # full 128-partition contraction covers both g
                                # blocks at once (partition = 64g + d)
                                nc.tensor.matmul(
                                    ps[:],
                                    ctx_v[:, rc, mm, :],
                                    wo[:, mm, 512 * oc:512 * (oc + 1)],
                                    start=(mm == 0), stop=(mm == 7))
                            outsb = mp.tile([128, 512], F32, tag="outsb", name="outsb")
                            nc.vector.tensor_tensor(
                                outsb[:], ps[:], bo_bc[:, 512 * oc:512 * (oc + 1)], ALU.add)
                            nc.sync.dma_start(
                                out_d[RP * p + 128 * rc:RP * p + 128 * (rc + 1),
                                      512 * oc:512 * (oc + 1)],
                                outsb[:])
    nc.compile()
    return nc


def _get_nc():
    if "nc" not in _cache:
        _cache["nc"] = _build()
    return _cache["nc"]


def kernel(x, Wq, bq, Wk, bk, Wv, bv, Wo, bo):
    x = np.asarray(x, np.float32)
    WqT = np.ascontiguousarray(np.asarray(Wq, np.float32).T)
    WkT = np.ascontiguousarray(np.asarray(Wk, np.float32).T)
    WvT = np.ascontiguousarray(np.asarray(Wv, np.float32).T)
    # woTre[64g + d, m, o] = Wo[o, 512g + 64m + d]
    WoTre = np.ascontiguousarray(
        np.asarray(Wo, np.float32).T.reshape(2, 8, 64, E).transpose(0, 2, 1, 3)
        .reshape(128, 8, E))
    bqT = np.ascontiguousarray(np.asarray(bq, np.float32).reshape(8, 128).T)
    bkT8 = np.ascontiguousarray((np.asarray(bk, np.float32) / 8.0).reshape(8, 128).T)
    bvrow = np.asarray(bv, np.float32).reshape(1, E)
    borow = np.asarray(bo, np.float32).reshape(1, E)

    in_maps = []
    for c in range(8):
        xTs = np.empty((E, R), np.float32)
        for p in range(2):
            h = 2 * c + p
            b_, mp_ = divmod(h, 8)
            xTs[:, RP * p:RP * (p + 1)] = x[b_, RP * mp_:RP * (mp_ + 1), :].T
        in_maps.append({
            "xT": np.ascontiguousarray(xTs), "wqT": WqT, "wkT": WkT,
            "wvT": WvT, "woTre": WoTre, "bqT": bqT, "bkT8": bkT8,
            "bvrow": bvrow, "borow": borow,
        })

    nc = _get_nc()
    res = run_bass_kernel_spmd(nc, in_maps, core_ids=list(range(8)))
    out = np.empty((2, 2048, E), np.float32)
    for c in range(8):
        o = res.results[c]["out"]
        for p in range(2):
            h = 2 * c + p
            b_, mp_ = divmod(h, 8)
            out[b_, RP * mp_:RP * (mp_ + 1), :] = o[RP * p:RP * (p + 1), :]
    return out
